# revision 85
# baseline (speedup 1.0000x reference)
"""Poker fused embedding kernel for 8x TRN2 NeuronCores (Bass/Tile).

Strategy (v3):
  - Host: shard batch across 8 cores (16 rows -> 16384 tokens/core).
    Sort each core's tokens into 128-token tiles by kind
    [card | action | context | CLS]; for every tile build a one-hot
    matrix [100, 128] (fp8) over a unified embedding table
    [base77+atype | street4 | rank13 | suit4 | actor2]; padding tokens
    get all-zero columns.
  - Device: per tile one fp8 matmul  onehot^T @ table -> PSUM f32.
    Action/context tiles additionally run the 17->256 MLP as one bf16
    matmul (bias via ones-row) + a tiny Gram matmul for the LayerNorm
    variance; rstd = Rsqrt(var+eps) in one ACT op; relu+scale on ACT,
    gather-add + bf16 store on DVE.  PSUM drains are balanced between
    ACT and DVE with a greedy busy-model (GpSimd has no PSUM access).
  - PE is kept gap-free: a short warm-up spin during the input DMA
    ramp lets the HAM clock-gate open (1.2 -> 2.4 GHz) early.
  - DMA: HWDGE rings only for latency-critical loads (sync + scalar),
    SWDGE (gpsimd Q7) for bulk/late loads and half the output flushes.
    Output is written bf16, transposed ([128 partitions, tile*256]) so
    each flush is a few KB per partition of contiguous DRAM.
  - Host: upcast bf16 -> f32 and scatter back to [B,S,D].
"""
import numpy as np
import ml_dtypes

import concourse.bacc as bacc
import concourse.tile as tile
from concourse import mybir
from concourse.bass_utils import run_bass_kernel_spmd
from concourse.tile_rust import add_dep_helper

F32 = mybir.dt.float32
BF16 = mybir.dt.bfloat16
FP8 = mybir.dt.float8e4
AF = mybir.ActivationFunctionType
ALU = mybir.AluOpType
NPBF = ml_dtypes.bfloat16
NPF8 = ml_dtypes.float8_e4m3

# problem constants
NBB = 16
D = 256
CARD_OFF = 8
ACTION_OFF = 60
CONTEXT_ID = 1
PAD = 76
NCTX = 16
B, S = 128, 1024
NCORES = 8
TPC = (B // NCORES) * S
TILE = 128

# unified table row layout; base rows 60..75 have atype_emb pre-added so
# an action token needs only its base row + street + actor
K = 100
R_STREET, R_RANK, R_SUIT, R_ACTOR = 77, 81, 94, 98
MRK = 17                   # MLP contraction rows: 16 features + ones

GRP = 4                    # tiles per card PSUM group (2 banks)
GRP_MLP = 2                # tiles per MLP PSUM group
LEAD = 5                   # leading card-only groups (consts still in flight)
POS_CLS = 7                # cls group position in the schedule
FT = 8                     # output slab size in tiles
N_SPIN = 6                 # PE warm-up matmuls
OH_SLICES = (3, 7, 11, 15, 19, 23, 27)   # oh slice boundaries (groups)
MLP_SQUEEZE = 0.85         # mlp groups finish by this fraction of slots


def _bf(a):
    return np.ascontiguousarray(np.asarray(a).astype(NPBF))


def _make_schedule(ct, at, xt):
    """Interleave card groups (GRP tiles) with MLP groups (GRP_MLP
    tiles); cls goes at POS_CLS so its long chain overlaps the rest."""
    cardg = []
    left = ct
    while left > 0:
        g = min(GRP, left)
        cardg.append(["card", g])
        left -= g
    mlpg = []
    for kind, n in (("act", at), ("ctx", xt)):
        left = n
        while left > 0:
            g = min(GRP_MLP, left)
            mlpg.append([kind, g])
            left -= g
    groups = []
    i = j = 0
    lead = min(LEAD, len(cardg))
    while i < lead:
        groups.append(cardg[i]); i += 1
    nrem_c = len(cardg) - lead
    while i < len(cardg) or j < len(mlpg):
        pc = (i - lead + 0.5) / max(nrem_c, 1)
        pm = (j + 0.5) / max(len(mlpg), 1) * MLP_SQUEEZE
        if j >= len(mlpg) or (i < len(cardg) and pc <= pm):
            groups.append(cardg[i]); i += 1
        else:
            groups.append(mlpg[j]); j += 1
    groups.insert(min(POS_CLS, len(groups)), ["cls", 1])
    t0 = 0
    seg_count = {"card": 0, "act": 0, "ctx": 0, "cls": 0}
    sched = []
    for kind, gn in groups:
        sched.append((kind, gn, t0, seg_count[kind]))
        t0 += gn
        seg_count[kind] += gn
    return sched, t0


def _build_host_data(token_ids, token_streets, card_ranks, card_suits,
                     action_actors, action_legal_masks, context_features):
    ids = token_ids.reshape(-1)
    streets = token_streets.reshape(-1)
    ranks = np.clip(card_ranks.reshape(-1), 0, 12)
    suits = np.clip(card_suits.reshape(-1), 0, 3)
    actors = np.clip(action_actors.reshape(-1), 0, 1)
    masks = action_legal_masks.reshape(-1, NBB)
    ctxf = context_features.reshape(-1, NCTX)

    cores = []
    for c in range(NCORES):
        lo = c * TPC
        idx = np.arange(lo, lo + TPC)
        cid = ids[idx]
        is_cls = (idx % S) == 0
        is_pad = cid < 0
        is_ctx = cid == CONTEXT_ID
        is_act = (cid >= ACTION_OFF) & (cid < PAD)
        rest = ~is_cls & ~is_pad
        cores.append(dict(
            cls=idx[is_cls],
            card=idx[rest & ~is_ctx & ~is_act],
            act=idx[rest & is_act],
            ctx=idx[rest & is_ctx]))

    ntiles = {k: max(-(-len(cc[k]) // TILE) for cc in cores)
              for k in ("card", "act", "ctx")}
    sched, nt = _make_schedule(ntiles["card"], ntiles["act"], ntiles["ctx"])

    def pad_seg(seg, n_tiles):
        out = np.full(n_tiles * TILE, -1, dtype=np.int64)
        out[: len(seg)] = seg
        return out

    per_core = []
    for c in range(NCORES):
        cc = cores[c]
        segs = {k: pad_seg(cc[k], ntiles[k]) for k in ("card", "act", "ctx")}
        segs["cls"] = pad_seg(cc["cls"], 1)
        slots = np.concatenate(
            [segs[kind][st * TILE:(st + gn) * TILE]
             for kind, gn, _, st in sched])
        valid = slots >= 0
        sl = np.where(valid, slots, 0)
        cid = np.where(valid, ids[sl], -1)
        live = valid & (cid >= 0)

        # one-hot [K, nt*TILE]
        n = nt * TILE
        cols = np.arange(n)
        oh = np.zeros((K, n), np.float32)
        lc, lid = cols[live], cid[live]
        oh[lid, lc] = 1.0
        oh[R_STREET + streets[sl][live], lc] = 1.0
        c_card = live & (cid >= CARD_OFF) & (cid < ACTION_OFF)
        oh[R_RANK + ranks[sl][c_card], cols[c_card]] = 1.0
        oh[R_SUIT + suits[sl][c_card], cols[c_card]] = 1.0
        c_act = live & (cid >= ACTION_OFF) & (cid < PAD)
        oh[R_ACTOR + actors[sl][c_act], cols[c_act]] = 1.0

        # act/ctx segment features (transposed) + ones row
        def featT(seg_slots, feats, nf):
            v = seg_slots >= 0
            s2 = np.where(v, seg_slots, 0)
            f = np.where(v[:, None], feats[s2], 0.0)
            return np.concatenate([f.T, v[None, :].astype(np.float32)])

        masksT = featT(segs["act"], masks, NBB)
        ctxT = featT(segs["ctx"], ctxf, NCTX)

        # CLS tile aux
        cls_sl = segs["cls"]
        cv = cls_sl >= 0
        csl = np.where(cv, cls_sl, 0)
        ccid = np.where(cv, ids[csl], -1)
        c_pad = ccid < 0
        mT_cls = featT(cls_sl, masks, NBB)
        xT_cls = featT(cls_sl, ctxf, NCTX)
        fT_cls = np.concatenate(
            [np.where(cv[:, None], ctxf[csl][:, :3], 0.0).T,
             cv[None, :].astype(np.float32)])
        cls_pack = np.zeros((MRK, 3 * TILE), np.float32)
        cls_pack[:, :TILE] = mT_cls
        cls_pack[:, TILE:2 * TILE] = xT_cls
        cls_pack[:4, 2 * TILE:] = fT_cls

        # untransposed augmented features, one 32-col block per MLP tile,
        # for the Gram-matrix variance reduction
        nta, ntx = ntiles["act"], ntiles["ctx"]
        featU = np.zeros((TILE, (nta + ntx + 3) * 32), np.float32)
        for bi in range(nta):
            featU[:, bi * 32:bi * 32 + MRK] = \
                masksT[:, bi * TILE:(bi + 1) * TILE].T
        for bi in range(ntx):
            fb = (nta + bi) * 32
            featU[:, fb:fb + MRK] = ctxT[:, bi * TILE:(bi + 1) * TILE].T
        u0 = (nta + ntx) * 32
        featU[:, u0:u0 + MRK] = mT_cls.T
        featU[:, u0 + 32:u0 + 32 + MRK] = xT_cls.T
        featU[:, u0 + 64:u0 + 64 + 4] = fT_cls.T
        # masks4: [act_mask, ctx_mask, ones, valid&!pad] per cls slot
        masks4 = np.stack(
            [((ccid >= ACTION_OFF) & (ccid < PAD)).astype(np.float32),
             (ccid == CONTEXT_ID).astype(np.float32),
             np.ones(TILE, np.float32),
             (~c_pad & cv).astype(np.float32)], axis=1)

        per_core.append(dict(
            slots=slots,
            oh=np.ascontiguousarray(oh.astype(NPF8)), masksT=_bf(masksT),
            ctxT=_bf(ctxT), cls_pack=_bf(cls_pack), featU=_bf(featU),
            masks4=np.ascontiguousarray(masks4.astype(np.float32))))
    return per_core, sched, nt, ntiles


def _fold_mean(W, b):
    """W' = W - rowmean, b' = b - mean(b): makes x@W'+b' == h - mean(h)."""
    W = np.asarray(W, np.float64)
    b = np.asarray(b, np.float64)
    return W - W.mean(-1, keepdims=True), b - b.mean()


def _build_tables(base_emb, street_emb, rank_emb, suit_emb, actor_emb,
                  atype_emb, legal_W, legal_b, ctx_W, ctx_b, cls_W, cls_b):
    base = np.asarray(base_emb[:77], np.float64).copy()
    base[ACTION_OFF:ACTION_OFF + NBB] += np.asarray(atype_emb, np.float64)
    t_all = np.concatenate([base, street_emb, rank_emb, suit_emb,
                            actor_emb]).astype(np.float32)
    assert t_all.shape == (K, D)
    t_all = np.ascontiguousarray(t_all.astype(NPF8))
    rhs = np.zeros((MRK, 3 * D), np.float32)
    # gmat: per-MLP Gram matrix G = W'W'^T/D, so that
    # var = sum((x @ G) * x)  (one DVE stt against SBUF features).
    gmat = np.zeros((MRK, 3 * 32), np.float32)
    for col, gcol, (W, b) in (
            (0, 0, _fold_mean(legal_W, legal_b)),
            (D, 32, _fold_mean(ctx_W, ctx_b)),
            (2 * D, 64, _fold_mean(cls_W, cls_b))):
        Wt = np.concatenate([W, b[None]])
        rhs[:Wt.shape[0], col:col + D] = Wt
        kr = Wt.shape[0]
        gmat[:kr, gcol:gcol + kr] = (Wt @ Wt.T) / D
    eye = np.eye(TILE, dtype=np.float32)
    return t_all, _bf(rhs), _bf(gmat), _bf(eye)


def _build_bass(sched, nt, ntiles):
    nc = bacc.Bacc("TRN2", target_bir_lowering=False)

    def din(name, shape, dt=BF16):
        return nc.dram_tensor(name, shape, dt, kind="ExternalInput")

    d_oh = din("oh", [K, nt * TILE], FP8)
    d_table = din("table", [K, D], FP8)
    d_rhs = din("rhs", [MRK, 3 * D])
    d_masksT = din("masksT", [MRK, ntiles["act"] * TILE])
    d_ctxT = din("ctxT", [MRK, ntiles["ctx"] * TILE])
    d_cls_pack = din("cls_pack", [MRK, 3 * TILE])
    d_masks4 = din("masks4", [TILE, 4], F32)
    d_eye = din("eye", [TILE, TILE])
    d_gmat = din("gmat", [MRK, 3 * 32])
    nfeat = (ntiles["act"] + ntiles["ctx"] + 3) * 32
    d_featU = din("featU", [TILE, nfeat])
    # compact per-dtype output spaces: card tiles -> fp8, rest -> bf16,
    # each numbered consecutively in processing order so slabs stay
    # contiguous and flushes stay big
    n8 = sum(gn for kind, gn, _, _ in sched if kind == "card")
    n16 = nt - n8
    d_out = nc.dram_tensor("out", [TILE, n16 * D], BF16,
                           kind="ExternalOutput")
    d_out8 = nc.dram_tensor("out8", [TILE, n8 * D], FP8,
                            kind="ExternalOutput")

    # oh slice boundaries in tiles
    obnd = [0] + [sum(g[1] for g in sched[:gg]) for gg in OH_SLICES] + [nt]

    with tile.TileContext(nc) as tc:
        with tc.tile_pool(name="const", bufs=1) as const_p, \
             tc.tile_pool(name="relu", bufs=3) as relu_p, \
             tc.tile_pool(name="rsc", bufs=3) as rsc_p, \
             tc.tile_pool(name="outp", bufs=4) as out_p, \
             tc.tile_pool(name="small", bufs=4) as small_p, \
             tc.tile_pool(name="p_out", bufs=3, space="PSUM") as po_p, \
             tc.tile_pool(name="p_h", bufs=2, space="PSUM") as ph_p:

            def load(d, shape, dt, eng):
                t = const_p.tile(shape, dt, tag=d.name)
                eng.dma_start(out=t, in_=d.ap())
                return t

            # --- PE warm-up: spin matmuls on a zero scratch tile so the
            # HAM clock gate opens during the input-DMA ramp.  Spins land
            # in the first po tile's bank (cleared again by the first real
            # gather), so no PSUM bank is wasted on them.
            spin_sb = const_p.tile([TILE, 512], BF16, tag="spin")
            nc.gpsimd.memset(spin_sb, 0.0)
            # dummy Sqrt as the first ACT op pins the sqrt table set (one
            # ACT_TABLE_LOAD instead of a mid-kernel switch)
            dummy = const_p.tile([TILE, 1], F32, tag="dummy")
            nc.scalar.activation(out=dummy, in_=spin_sb[:, 0:1],
                                 func=AF.Sqrt)
            po0 = po_p.tile([TILE, GRP * D], F32, tag="po")
            prev = None
            for i in range(N_SPIN):
                mm = nc.tensor.matmul(po0[:, :512], lhsT=spin_sb[:, :TILE],
                                      rhs=spin_sb, start=True, stop=True,
                                      skip_group_check=True)
                if prev is not None:
                    add_dep_helper(mm.ins, prev.ins, sync=False,
                                   reason="spin order")
                prev = mm
            spin_last = prev

            # --- input loads.  Ring FIFOs drain slowly (~45-90 GB/s), so
            # the one-hot is cut into need-ordered slices spread across the
            # three rings; per-ring arrival order matches need order.
            oh_t = []
            for si in range(len(obnd) - 1):
                w = obnd[si + 1] - obnd[si]
                t = const_p.tile([K, w * TILE], FP8, tag=f"oh{si}",
                                 name=f"oh{si}")
                oh_t.append(t)

            def loadp(t, dsl, engs):
                n = len(engs)
                step = -(-K // n)
                for i, eng in enumerate(engs):
                    p0, p1 = i * step, min((i + 1) * step, K)
                    eng.dma_start(out=t[p0:p1, :], in_=dsl[p0:p1, :])

            def oh_load(si, engs):
                a, b = obnd[si], obnd[si + 1]
                loadp(oh_t[si], d_oh.ap()[:, a * TILE:b * TILE], engs)

            oh_load(0, [nc.sync, nc.scalar, nc.gpsimd])
            # table+rhs ride the scalar ring so slice s1 streams on sync
            # immediately behind s0 (kills the group-3 stall)
            t_table = load(d_table, [K, D], FP8, nc.scalar)
            t_rhs = load(d_rhs, [MRK, 3 * D], BF16, nc.scalar)
            t_masksT = load(d_masksT, [MRK, ntiles["act"] * TILE], BF16,
                            nc.gpsimd)
            oh_load(1, [nc.sync])
            oh_load(2, [nc.scalar])
            t_gmat = load(d_gmat, [MRK, 3 * 32], BF16, nc.gpsimd)
            oh_load(3, [nc.gpsimd])
            oh_load(4, [nc.sync])
            t_featU = const_p.tile([TILE, nfeat], BF16, tag="featU")
            nc.sync.dma_start(out=t_featU[:64, :], in_=d_featU.ap()[:64, :])
            nc.scalar.dma_start(out=t_featU[64:, :],
                                in_=d_featU.ap()[64:, :])
            t_cls_pack = load(d_cls_pack, [MRK, 3 * TILE], BF16, nc.gpsimd)
            t_masks4 = load(d_masks4, [TILE, 4], F32, nc.gpsimd)
            oh_load(5, [nc.scalar])
            t_eye = load(d_eye, [TILE, TILE], BF16, nc.gpsimd)
            t_ctxT = load(d_ctxT, [MRK, ntiles["ctx"] * TILE], BF16,
                          nc.gpsimd)
            oh_load(6, [nc.sync])
            oh_load(7, [nc.scalar, nc.gpsimd])

            eps_t = const_p.tile([TILE, 1], F32, tag="eps")
            nc.vector.memset(eps_t, 1e-5)

            # --- engine busy model for drain balancing ---
            busy = {"A": 0.0, "V": 0.0}
            A_FIX, A_NS = 160.0, 0.95
            V_FIX, V_NS = 130.0, 1.05

            def pick(cols, v_extra=0.0):
                ca = busy["A"] + A_FIX + A_NS * cols
                cv = busy["V"] + V_FIX + V_NS * cols + v_extra
                if ca <= cv:
                    busy["A"] = ca
                    return "A"
                busy["V"] = cv
                return "V"

            def charge(eng, cols):
                if eng == "A":
                    busy["A"] += A_FIX + A_NS * cols
                else:
                    busy["V"] += V_FIX + V_NS * cols

            def get_oh(t0):
                for si in range(len(obnd) - 1):
                    if t0 < obnd[si + 1]:
                        return oh_t[si][:, (t0 - obnd[si]) * TILE:]
                raise AssertionError(t0)

            def gather(po, oh_sl, gn, prev=None):
                for i in range(gn):
                    mm = nc.tensor.matmul(
                        po[:, i * D:(i + 1) * D],
                        lhsT=oh_sl[:, i * TILE:(i + 1) * TILE],
                        rhs=t_table, start=(i % 2 == 0), stop=True,
                        skip_group_check=True)
                    if prev is not None:
                        add_dep_helper(mm.ins, prev.ins, sync=False,
                                       reason="psum bank order")
                    prev = mm
                return prev

            # --- output slab / flush machinery (dual dtype: card tiles go
            # out fp8, mlp/cls tiles bf16) ---
            sbuf8 = {"tile": None, "t0": 0, "cols": 0}
            sbuf16 = {"tile": None, "t0": 0, "cols": 0}
            flush_rr = [0]
            FT8 = 2 * FT

            def _flush(sb, dten):
                if sb["tile"] is None:
                    return
                t, t0, cols = sb["tile"], sb["t0"], sb["cols"]
                sb["tile"] = None
                rings = [nc.gpsimd, nc.sync, nc.scalar]
                eng = rings[flush_rr[0] % 3]
                flush_rr[0] += 1
                eng.dma_start(out=dten.ap()[:, t0 * D:t0 * D + cols],
                              in_=t[:, :cols])

            def _slot(sb, dten, t0, ncols, cap, mk):
                if sb["tile"] is not None and (
                        sb["t0"] * D + sb["cols"] != t0 * D
                        or sb["cols"] + ncols > cap * D):
                    _flush(sb, dten)
                if sb["tile"] is None:
                    sb.update(tile=mk(), t0=t0, cols=0)
                off = sb["cols"]
                sb["cols"] += ncols
                return sb["tile"], off

            def store_slot8(t0, ncols):
                return _slot(sbuf8, d_out8, t0, ncols, FT8,
                             lambda: out_p.tile([TILE, FT8 * D], FP8,
                                                tag="o8", name="o8t"))

            def store_slot(t0, ncols):
                return _slot(sbuf16, d_out, t0, ncols, FT,
                             lambda: out_p.tile([TILE, FT * D], BF16,
                                                tag="o", name="o16t"))

            def maybe_flush():
                if sbuf8["tile"] is not None and sbuf8["cols"] >= FT8 * D:
                    _flush(sbuf8, d_out8)
                if sbuf16["tile"] is not None and sbuf16["cols"] >= FT * D:
                    _flush(sbuf16, d_out)

            def flush_store():
                _flush(sbuf8, d_out8)
                _flush(sbuf16, d_out)

            # --- per-kind drain / finish ops ---
            def copy_store(po, t0, gn):
                cols = gn * D
                o_sb, off = store_slot8(t0, cols)
                if pick(cols) == "A":
                    nc.scalar.activation(out=o_sb[:, off:off + cols],
                                         in_=po[:, :cols], func=AF.Copy)
                else:
                    nc.vector.tensor_copy(out=o_sb[:, off:off + cols],
                                          in_=po[:, :cols])
                maybe_flush()

            def mlp_mms(po, ph, phS, mms, prev_mm):
                for i, (lhsT_sl, rhs_w, rhs_g, _u) in enumerate(mms):
                    mm = nc.tensor.matmul(
                        ph[:, i * D:(i + 1) * D],
                        lhsT=lhsT_sl, rhs=rhs_w,
                        start=(i % 2 == 0), stop=True,
                        skip_group_check=True)
                    if prev_mm is not None:
                        add_dep_helper(mm.ins, prev_mm.ins, sync=False,
                                       reason="psum bank order")
                    mm2 = nc.tensor.matmul(
                        phS[:, i * 32:(i + 1) * 32],
                        lhsT=lhsT_sl, rhs=rhs_g,
                        start=(i == 0), stop=True,
                        skip_group_check=True)
                    add_dep_helper(mm2.ins, mm.ins, sync=False,
                                   reason="psum bank order")
                    prev_mm = mm2
                return prev_mm

            def mlp_var_rstd(phS, mms, vr, rr):
                """var = sum((x@G) * x) per tile, rr = 1/sqrt(var+eps)."""
                gn = len(mms)
                scr = small_p.tile([TILE, GRP * 32], F32, tag="scr")
                for i, (_l, _w, _g, ublock) in enumerate(mms):
                    kr = ublock[1]
                    nc.vector.scalar_tensor_tensor(
                        out=scr[:, i * 32:i * 32 + kr],
                        in0=phS[:, i * 32:i * 32 + kr], scalar=1.0,
                        in1=t_featU[:, ublock[0]:ublock[0] + kr],
                        op0=ALU.mult, op1=ALU.mult,
                        accum_out=vr[:, i:i + 1])
                    charge("V", kr)
                sd = small_p.tile([TILE, GRP], F32, tag="sd")
                nc.scalar.activation(out=sd[:, :gn], in_=vr[:, :gn],
                                     func=AF.Sqrt, bias=eps_t)
                busy["A"] += A_FIX + gn
                nc.vector.reciprocal(out=rr[:, :gn], in_=sd[:, :gn])
                busy["V"] += V_FIX + gn

            def mlp_relus(ph, gn, rsc):
                # plain relu (no scale) right after the h' matmuls, one op
                # per group; the rstd scale rides the stt's scalar AP
                cols = gn * D
                if pick(cols) == "A":
                    nc.scalar.activation(out=rsc[:, :cols],
                                         in_=ph[:, :cols], func=AF.Relu)
                else:
                    nc.vector.tensor_scalar(
                        out=rsc[:, :cols], in0=ph[:, :cols],
                        scalar1=0.0, scalar2=None, op0=ALU.max)

            def stage_b_mlp(t0, gn, po, rr, rsc):
                o_sb, off = store_slot(t0, gn * D)
                for i in range(gn):
                    # out = relu(h') * rstd + gather
                    nc.vector.scalar_tensor_tensor(
                        out=o_sb[:, off + i * D:off + (i + 1) * D],
                        in0=rsc[:, i * D:(i + 1) * D],
                        scalar=rr[:, i:i + 1],
                        in1=po[:, i * D:(i + 1) * D],
                        op0=ALU.mult, op1=ALU.add)
                    charge("V", D)
                maybe_flush()

            from collections import defaultdict
            tasks = defaultdict(list)
            onext = {"o8": 0, "o16": 0}

            for gi, (kind, gn, t0, st) in enumerate(sched):
                for fn in tasks.pop(gi, ()):
                    fn()
                oh_sl = get_oh(t0)
                po = po0 if gi == 0 else po_p.tile([TILE, GRP * D], F32,
                                                   tag="po")
                last_mm = gather(po, oh_sl, gn,
                                 prev=spin_last if gi == 0 else None)
                okey = "o8" if kind == "card" else "o16"
                oslot = onext[okey]
                onext[okey] += gn

                if kind == "card":
                    tasks[gi + 1].append(
                        lambda po=po, t0=oslot, gn=gn:
                        copy_store(po, t0, gn))
                elif kind in ("act", "ctx"):
                    lhsT = t_masksT if kind == "act" else t_ctxT
                    rhs_w = (t_rhs[:, :D] if kind == "act"
                             else t_rhs[:, D:2 * D])
                    rhs_g = (t_gmat[:, 0:32] if kind == "act"
                             else t_gmat[:, 32:64])
                    u_base = 0 if kind == "act" else ntiles["act"] * 32
                    ph = po[:, GRP_MLP * D:2 * GRP_MLP * D]
                    phS = ph_p.tile([TILE, GRP * 32], F32, tag="phS")
                    mms = [(lhsT[:, (st + i) * TILE:(st + i + 1) * TILE],
                            rhs_w, rhs_g, ((u_base + (st + i) * 32), MRK))
                           for i in range(gn)]
                    mlp_mms(po, ph, phS, mms, last_mm)
                    vr = small_p.tile([TILE, 8], F32, tag="vr")
                    rr = small_p.tile([TILE, GRP], F32, tag="rr")
                    rsc = rsc_p.tile([TILE, GRP_MLP * D], BF16, tag="rsc")
                    mlp_relus(ph, gn, rsc)
                    mlp_var_rstd(phS, mms, vr, rr)
                    tasks[gi + 1].append(
                        lambda t0=oslot, gn=gn, po=po, rr=rr,
                        rsc=rsc: stage_b_mlp(t0, gn, po, rr, rsc))
                else:  # cls
                    phS = ph_p.tile([TILE, GRP * 32], F32, tag="phS")
                    u0 = (ntiles["act"] + ntiles["ctx"]) * 32
                    mms = [
                        (t_cls_pack[:, 0:TILE], t_rhs[:, :D],
                         t_gmat[:, 0:32], (u0, MRK)),
                        (t_cls_pack[:, TILE:2 * TILE], t_rhs[:, D:2 * D],
                         t_gmat[:, 32:64], (u0 + 32, MRK)),
                        (t_cls_pack[0:4, 2 * TILE:3 * TILE],
                         t_rhs[0:4, 2 * D:3 * D],
                         t_gmat[0:4, 64:96], (u0 + 64, 4)),
                    ]
                    hsl = [slice(2 * D, 3 * D), slice(3 * D, 4 * D),
                           slice(D, 2 * D)]
                    prev_mm = last_mm
                    for i, (lhsT_sl, rhs_w, rhs_g, _u) in enumerate(mms):
                        mm = nc.tensor.matmul(
                            po[:, hsl[i]], lhsT=lhsT_sl, rhs=rhs_w,
                            start=(i == 0), stop=True,
                            skip_group_check=True)
                        add_dep_helper(mm.ins, prev_mm.ins, sync=False,
                                       reason="psum bank order")
                        mm2 = nc.tensor.matmul(
                            phS[:, i * 32:(i + 1) * 32],
                            lhsT=lhsT_sl, rhs=rhs_g,
                            start=(i == 0), stop=True,
                            skip_group_check=True)
                        add_dep_helper(mm2.ins, mm.ins, sync=False,
                                       reason="psum bank order")
                        prev_mm = mm2
                    vr = small_p.tile([TILE, 8], F32, tag="vr")
                    rr = small_p.tile([TILE, GRP], F32, tag="rr")
                    mlp_var_rstd(phS, mms, vr, rr)
                    mr = small_p.tile([TILE, 3], F32, tag="mr")
                    nc.vector.tensor_tensor(
                        out=mr[:, 0:3], in0=t_masks4[:, 0:3],
                        in1=rr[:, 0:3], op=ALU.mult)
                    busy["V"] += V_FIX + 3

                    relu_t = relu_p.tile([TILE, 3 * D], BF16, tag="relu")

                    def cls_relu(po=po, hsl=hsl, mr=mr, relu_t=relu_t):
                        for i in range(3):
                            # relu((mask*rstd) * h') = mask*rstd*relu(h')
                            if pick(D) == "A":
                                nc.scalar.activation(
                                    out=relu_t[:, i * D:(i + 1) * D],
                                    in_=po[:, hsl[i]], func=AF.Relu,
                                    scale=mr[:, i:i + 1])
                            else:
                                nc.vector.tensor_scalar(
                                    out=relu_t[:, i * D:(i + 1) * D],
                                    in0=po[:, hsl[i]],
                                    scalar1=mr[:, i:i + 1], scalar2=0.0,
                                    op0=ALU.mult, op1=ALU.max)

                    def cls_acc(po=po, relu_t=relu_t):
                        # accumulate the three relu terms onto the gather
                        # PSUM via identity matmuls
                        prev = None
                        for i in range(3):
                            mm = nc.tensor.matmul(
                                po[:, :D], lhsT=t_eye,
                                rhs=relu_t[:, i * D:(i + 1) * D],
                                start=False, stop=(i == 2),
                                skip_group_check=True)
                            if prev is not None:
                                add_dep_helper(mm.ins, prev.ins,
                                               sync=False,
                                               reason="psum acc order")
                            prev = mm

                    def cls_drain(t0=oslot, po=po):
                        o_sb, off = store_slot(t0, D)
                        if pick(D) == "A":
                            nc.scalar.activation(
                                out=o_sb[:, off:off + D], in_=po[:, :D],
                                func=AF.Copy, scale=t_masks4[:, 3:4])
                        else:
                            nc.vector.tensor_scalar(
                                out=o_sb[:, off:off + D], in0=po[:, :D],
                                scalar1=t_masks4[:, 3:4], scalar2=None,
                                op0=ALU.mult)
                        maybe_flush()

                    tasks[gi + 1].append(cls_relu)
                    tasks[gi + 2].append(cls_acc)
                    tasks[gi + 2].append(cls_drain)

            for i in sorted(tasks):
                for fn in tasks[i]:
                    fn()
            flush_store()

    if not nc.is_finalized():
        nc.finalize()
    return nc


def kernel(token_ids, token_streets, card_ranks, card_suits, action_actors,
           action_legal_masks, context_features,
           base_emb, street_emb, rank_emb, suit_emb, actor_emb, atype_emb,
           legal_W, legal_b, legal_g, legal_be,
           cls_W, cls_b, cls_g, cls_be,
           ctx_W, ctx_b, ctx_g, ctx_be, _trace=False):
    per_core, sched, nt, ntiles = _build_host_data(
        np.asarray(token_ids), np.asarray(token_streets),
        np.asarray(card_ranks), np.asarray(card_suits),
        np.asarray(action_actors), np.asarray(action_legal_masks),
        np.asarray(context_features))

    for g, be in ((legal_g, legal_be), (cls_g, cls_be), (ctx_g, ctx_be)):
        assert np.allclose(np.asarray(g), 1.0) and np.allclose(
            np.asarray(be), 0.0), "non-trivial LN affine not supported"

    t_all, rhs, gmat, eye = _build_tables(
        np.asarray(base_emb), np.asarray(street_emb), np.asarray(rank_emb),
        np.asarray(suit_emb), np.asarray(actor_emb), np.asarray(atype_emb),
        np.asarray(legal_W), np.asarray(legal_b), np.asarray(ctx_W),
        np.asarray(ctx_b), np.asarray(cls_W), np.asarray(cls_b))

    nc = _build_bass(sched, nt, ntiles)

    shared = dict(table=t_all, rhs=rhs, gmat=gmat, eye=eye)
    in_maps = []
    for c in range(NCORES):
        pc = per_core[c]
        im = dict(shared)
        im.update(oh=pc["oh"], masksT=pc["masksT"], ctxT=pc["ctxT"],
                  cls_pack=pc["cls_pack"], featU=pc["featU"],
                  masks4=pc["masks4"])
        in_maps.append(im)

    res = run_bass_kernel_spmd(nc, in_maps, core_ids=list(range(NCORES)),
                               trace=_trace)
    if _trace:
        print(f"HW exec time: {res.exec_time_ns} ns")
        print(f"mean exec time: {res.mean_exec_time_ns} ns")
        if res.instructions_and_trace:
            print("trace:", res.instructions_and_trace[1])

    # per-tile output mapping: card tiles -> (out8, slot), rest -> (out, slot)
    src8 = np.full(nt, -1, np.int64)
    src16 = np.full(nt, -1, np.int64)
    c8 = c16 = 0
    for kind, gn, t0, _ in sched:
        if kind == "card":
            src8[t0:t0 + gn] = np.arange(c8, c8 + gn)
            c8 += gn
        else:
            src16[t0:t0 + gn] = np.arange(c16, c16 + gn)
            c16 += gn

    full = np.zeros((B * S, D), np.float32)
    for c in range(NCORES):
        o16 = np.asarray(res.results[c]["out"]).astype(np.float32)
        o8 = np.asarray(res.results[c]["out8"]).astype(np.float32)
        o16 = o16.reshape(TILE, c16, D).transpose(1, 0, 2)
        o8 = o8.reshape(TILE, c8, D).transpose(1, 0, 2)
        rows = np.empty((nt, TILE, D), np.float32)
        rows[src8 >= 0] = o8[src8[src8 >= 0]]
        rows[src16 >= 0] = o16[src16[src16 >= 0]]
        rows = rows.reshape(-1, D)
        slots = per_core[c]["slots"]
        valid = slots >= 0
        full[slots[valid]] = rows[valid]
    return full.reshape(B, S, D)


# revision 86
# speedup vs baseline: 1.0202x; 1.0202x over previous
"""Poker fused embedding kernel for 8x TRN2 NeuronCores (Bass/Tile).

Strategy (v3):
  - Host: shard batch across 8 cores (16 rows -> 16384 tokens/core).
    Sort each core's tokens into 128-token tiles by kind
    [card | action | context | CLS]; for every tile build a one-hot
    matrix [100, 128] (fp8) over a unified embedding table
    [base77+atype | street4 | rank13 | suit4 | actor2]; padding tokens
    get all-zero columns.
  - Device: per tile one fp8 matmul  onehot^T @ table -> PSUM f32.
    Action/context tiles additionally run the 17->256 MLP as one bf16
    matmul (bias via ones-row) + a tiny Gram matmul for the LayerNorm
    variance; rstd = Rsqrt(var+eps) in one ACT op; relu+scale on ACT,
    gather-add + bf16 store on DVE.  PSUM drains are balanced between
    ACT and DVE with a greedy busy-model (GpSimd has no PSUM access).
  - PE is kept gap-free: a short warm-up spin during the input DMA
    ramp lets the HAM clock-gate open (1.2 -> 2.4 GHz) early.
  - DMA: HWDGE rings only for latency-critical loads (sync + scalar),
    SWDGE (gpsimd Q7) for bulk/late loads and half the output flushes.
    Output is written bf16, transposed ([128 partitions, tile*256]) so
    each flush is a few KB per partition of contiguous DRAM.
  - Host: upcast bf16 -> f32 and scatter back to [B,S,D].
"""
import numpy as np
import ml_dtypes

import concourse.bacc as bacc
import concourse.tile as tile
from concourse import mybir
from concourse.bass_utils import run_bass_kernel_spmd
from concourse.tile_rust import add_dep_helper

F32 = mybir.dt.float32
BF16 = mybir.dt.bfloat16
FP8 = mybir.dt.float8e4
AF = mybir.ActivationFunctionType
ALU = mybir.AluOpType
NPBF = ml_dtypes.bfloat16
NPF8 = ml_dtypes.float8_e4m3

# problem constants
NBB = 16
D = 256
CARD_OFF = 8
ACTION_OFF = 60
CONTEXT_ID = 1
PAD = 76
NCTX = 16
B, S = 128, 1024
NCORES = 8
TPC = (B // NCORES) * S
TILE = 128

# unified table row layout; base rows 60..75 have atype_emb pre-added so
# an action token needs only its base row + street + actor
K = 100
R_STREET, R_RANK, R_SUIT, R_ACTOR = 77, 81, 94, 98
MRK = 17                   # MLP contraction rows: 16 features + ones

GRP = 4                    # tiles per card PSUM group (2 banks)
GRP_MLP = 2                # tiles per MLP PSUM group
LEAD = 5                   # leading card-only groups (consts still in flight)
POS_CLS = 7                # cls group position in the schedule
FT = 8                     # output slab size in tiles
N_SPIN = 6                 # PE warm-up matmuls
OH_SLICES = (3, 7, 11, 15, 19, 23, 27)   # oh slice boundaries (groups)
MLP_SQUEEZE = 0.85         # mlp groups finish by this fraction of slots


def _bf(a):
    return np.ascontiguousarray(np.asarray(a).astype(NPBF))


def _make_schedule(ct, at, xt):
    """Interleave card groups (GRP tiles) with MLP groups (GRP_MLP
    tiles); cls goes at POS_CLS so its long chain overlaps the rest."""
    cardg = []
    left = ct
    while left > 0:
        g = min(GRP, left)
        cardg.append(["card", g])
        left -= g
    mlpg = []
    for kind, n in (("act", at), ("ctx", xt)):
        left = n
        while left > 0:
            g = min(GRP_MLP, left)
            mlpg.append([kind, g])
            left -= g
    groups = []
    i = j = 0
    lead = min(LEAD, len(cardg))
    while i < lead:
        groups.append(cardg[i]); i += 1
    nrem_c = len(cardg) - lead
    while i < len(cardg) or j < len(mlpg):
        pc = (i - lead + 0.5) / max(nrem_c, 1)
        pm = (j + 0.5) / max(len(mlpg), 1) * MLP_SQUEEZE
        if j >= len(mlpg) or (i < len(cardg) and pc <= pm):
            groups.append(cardg[i]); i += 1
        else:
            groups.append(mlpg[j]); j += 1
    groups.insert(min(POS_CLS, len(groups)), ["cls", 1])
    t0 = 0
    seg_count = {"card": 0, "act": 0, "ctx": 0, "cls": 0}
    sched = []
    for kind, gn in groups:
        sched.append((kind, gn, t0, seg_count[kind]))
        t0 += gn
        seg_count[kind] += gn
    return sched, t0


def _build_host_data(token_ids, token_streets, card_ranks, card_suits,
                     action_actors, action_legal_masks, context_features):
    ids = token_ids.reshape(-1)
    streets = token_streets.reshape(-1)
    ranks = np.clip(card_ranks.reshape(-1), 0, 12)
    suits = np.clip(card_suits.reshape(-1), 0, 3)
    actors = np.clip(action_actors.reshape(-1), 0, 1)
    masks = action_legal_masks.reshape(-1, NBB)
    ctxf = context_features.reshape(-1, NCTX)

    cores = []
    for c in range(NCORES):
        lo = c * TPC
        idx = np.arange(lo, lo + TPC)
        cid = ids[idx]
        is_cls = (idx % S) == 0
        is_pad = cid < 0
        is_ctx = cid == CONTEXT_ID
        is_act = (cid >= ACTION_OFF) & (cid < PAD)
        rest = ~is_cls & ~is_pad
        cores.append(dict(
            cls=idx[is_cls],
            card=idx[rest & ~is_ctx & ~is_act],
            act=idx[rest & is_act],
            ctx=idx[rest & is_ctx]))

    ntiles = {k: max(-(-len(cc[k]) // TILE) for cc in cores)
              for k in ("card", "act", "ctx")}
    sched, nt = _make_schedule(ntiles["card"], ntiles["act"], ntiles["ctx"])

    def pad_seg(seg, n_tiles):
        out = np.full(n_tiles * TILE, -1, dtype=np.int64)
        out[: len(seg)] = seg
        return out

    per_core = []
    for c in range(NCORES):
        cc = cores[c]
        segs = {k: pad_seg(cc[k], ntiles[k]) for k in ("card", "act", "ctx")}
        segs["cls"] = pad_seg(cc["cls"], 1)
        slots = np.concatenate(
            [segs[kind][st * TILE:(st + gn) * TILE]
             for kind, gn, _, st in sched])
        valid = slots >= 0
        sl = np.where(valid, slots, 0)
        cid = np.where(valid, ids[sl], -1)
        live = valid & (cid >= 0)

        # one-hot [K, nt*TILE]
        n = nt * TILE
        cols = np.arange(n)
        oh = np.zeros((K, n), np.float32)
        lc, lid = cols[live], cid[live]
        oh[lid, lc] = 1.0
        oh[R_STREET + streets[sl][live], lc] = 1.0
        c_card = live & (cid >= CARD_OFF) & (cid < ACTION_OFF)
        oh[R_RANK + ranks[sl][c_card], cols[c_card]] = 1.0
        oh[R_SUIT + suits[sl][c_card], cols[c_card]] = 1.0
        c_act = live & (cid >= ACTION_OFF) & (cid < PAD)
        oh[R_ACTOR + actors[sl][c_act], cols[c_act]] = 1.0

        # act/ctx segment features (transposed) + ones row
        def featT(seg_slots, feats, nf):
            v = seg_slots >= 0
            s2 = np.where(v, seg_slots, 0)
            f = np.where(v[:, None], feats[s2], 0.0)
            return np.concatenate([f.T, v[None, :].astype(np.float32)])

        masksT = featT(segs["act"], masks, NBB)
        ctxT = featT(segs["ctx"], ctxf, NCTX)

        # CLS tile aux
        cls_sl = segs["cls"]
        cv = cls_sl >= 0
        csl = np.where(cv, cls_sl, 0)
        ccid = np.where(cv, ids[csl], -1)
        c_pad = ccid < 0
        mT_cls = featT(cls_sl, masks, NBB)
        xT_cls = featT(cls_sl, ctxf, NCTX)
        fT_cls = np.concatenate(
            [np.where(cv[:, None], ctxf[csl][:, :3], 0.0).T,
             cv[None, :].astype(np.float32)])
        cls_pack = np.zeros((MRK, 3 * TILE), np.float32)
        cls_pack[:, :TILE] = mT_cls
        cls_pack[:, TILE:2 * TILE] = xT_cls
        cls_pack[:4, 2 * TILE:] = fT_cls

        # untransposed augmented features, one 32-col block per MLP tile,
        # for the Gram-matrix variance reduction
        nta, ntx = ntiles["act"], ntiles["ctx"]
        featU = np.zeros((TILE, (nta + ntx + 3) * 32), np.float32)
        for bi in range(nta):
            featU[:, bi * 32:bi * 32 + MRK] = \
                masksT[:, bi * TILE:(bi + 1) * TILE].T
        for bi in range(ntx):
            fb = (nta + bi) * 32
            featU[:, fb:fb + MRK] = ctxT[:, bi * TILE:(bi + 1) * TILE].T
        u0 = (nta + ntx) * 32
        featU[:, u0:u0 + MRK] = mT_cls.T
        featU[:, u0 + 32:u0 + 32 + MRK] = xT_cls.T
        featU[:, u0 + 64:u0 + 64 + 4] = fT_cls.T
        # masks4: [act_mask, ctx_mask, ones, valid&!pad] per cls slot
        masks4 = np.stack(
            [((ccid >= ACTION_OFF) & (ccid < PAD)).astype(np.float32),
             (ccid == CONTEXT_ID).astype(np.float32),
             np.ones(TILE, np.float32),
             (~c_pad & cv).astype(np.float32)], axis=1)

        per_core.append(dict(
            slots=slots,
            oh=np.ascontiguousarray(oh.astype(NPF8)), masksT=_bf(masksT),
            ctxT=_bf(ctxT), cls_pack=_bf(cls_pack), featU=_bf(featU),
            masks4=np.ascontiguousarray(masks4.astype(np.float32))))
    return per_core, sched, nt, ntiles


def _fold_mean(W, b):
    """W' = W - rowmean, b' = b - mean(b): makes x@W'+b' == h - mean(h)."""
    W = np.asarray(W, np.float64)
    b = np.asarray(b, np.float64)
    return W - W.mean(-1, keepdims=True), b - b.mean()


def _build_tables(base_emb, street_emb, rank_emb, suit_emb, actor_emb,
                  atype_emb, legal_W, legal_b, ctx_W, ctx_b, cls_W, cls_b):
    base = np.asarray(base_emb[:77], np.float64).copy()
    base[ACTION_OFF:ACTION_OFF + NBB] += np.asarray(atype_emb, np.float64)
    t_all = np.concatenate([base, street_emb, rank_emb, suit_emb,
                            actor_emb]).astype(np.float32)
    assert t_all.shape == (K, D)
    t_all = np.ascontiguousarray(t_all.astype(NPF8))
    rhs = np.zeros((MRK, 3 * D), np.float32)
    # gmat: per-MLP Gram matrix G = W'W'^T/D, so that
    # var = sum((x @ G) * x)  (one DVE stt against SBUF features).
    gmat = np.zeros((MRK, 3 * 32), np.float32)
    for col, gcol, (W, b) in (
            (0, 0, _fold_mean(legal_W, legal_b)),
            (D, 32, _fold_mean(ctx_W, ctx_b)),
            (2 * D, 64, _fold_mean(cls_W, cls_b))):
        Wt = np.concatenate([W, b[None]])
        rhs[:Wt.shape[0], col:col + D] = Wt
        kr = Wt.shape[0]
        gmat[:kr, gcol:gcol + kr] = (Wt @ Wt.T) / D
    eye = np.eye(TILE, dtype=np.float32)
    return t_all, _bf(rhs), _bf(gmat), _bf(eye)


def _build_bass(sched, nt, ntiles):
    nc = bacc.Bacc("TRN2", target_bir_lowering=False)

    def din(name, shape, dt=BF16):
        return nc.dram_tensor(name, shape, dt, kind="ExternalInput")

    d_oh = din("oh", [K, nt * TILE], FP8)
    d_table = din("table", [K, D], FP8)
    d_rhs = din("rhs", [MRK, 3 * D])
    d_masksT = din("masksT", [MRK, ntiles["act"] * TILE])
    d_ctxT = din("ctxT", [MRK, ntiles["ctx"] * TILE])
    d_cls_pack = din("cls_pack", [MRK, 3 * TILE])
    d_masks4 = din("masks4", [TILE, 4], F32)
    d_eye = din("eye", [TILE, TILE])
    d_gmat = din("gmat", [MRK, 3 * 32])
    nfeat = (ntiles["act"] + ntiles["ctx"] + 3) * 32
    d_featU = din("featU", [TILE, nfeat])
    # compact per-dtype output spaces: card tiles -> fp8, rest -> bf16,
    # each numbered consecutively in processing order so slabs stay
    # contiguous and flushes stay big
    n8 = sum(gn for kind, gn, _, _ in sched if kind == "card")
    n16 = nt - n8
    d_out = nc.dram_tensor("out", [TILE, n16 * D], BF16,
                           kind="ExternalOutput")
    d_out8 = nc.dram_tensor("out8", [TILE, n8 * D], FP8,
                            kind="ExternalOutput")

    # oh slice boundaries in tiles
    obnd = [0] + [sum(g[1] for g in sched[:gg]) for gg in OH_SLICES] + [nt]

    with tile.TileContext(nc) as tc:
        with tc.tile_pool(name="const", bufs=1) as const_p, \
             tc.tile_pool(name="relu", bufs=3) as relu_p, \
             tc.tile_pool(name="rsc", bufs=3) as rsc_p, \
             tc.tile_pool(name="outp", bufs=4) as out_p, \
             tc.tile_pool(name="small", bufs=4) as small_p, \
             tc.tile_pool(name="p_out", bufs=3, space="PSUM") as po_p, \
             tc.tile_pool(name="p_h", bufs=2, space="PSUM") as ph_p:

            def load(d, shape, dt, eng):
                t = const_p.tile(shape, dt, tag=d.name)
                eng.dma_start(out=t, in_=d.ap())
                return t

            # --- PE warm-up: spin matmuls on a zero scratch tile so the
            # HAM clock gate opens during the input-DMA ramp.  Spins land
            # in the first po tile's bank (cleared again by the first real
            # gather), so no PSUM bank is wasted on them.
            spin_sb = const_p.tile([TILE, 512], BF16, tag="spin")
            nc.gpsimd.memset(spin_sb, 0.0)
            # dummy Sqrt as the first ACT op pins the sqrt table set (one
            # ACT_TABLE_LOAD instead of a mid-kernel switch)
            dummy = const_p.tile([TILE, 1], F32, tag="dummy")
            nc.scalar.activation(out=dummy, in_=spin_sb[:, 0:1],
                                 func=AF.Sqrt)
            po0 = po_p.tile([TILE, GRP * D], F32, tag="po")
            prev = None
            for i in range(N_SPIN):
                mm = nc.tensor.matmul(po0[:, :512], lhsT=spin_sb[:, :TILE],
                                      rhs=spin_sb, start=True, stop=True,
                                      skip_group_check=True)
                if prev is not None:
                    add_dep_helper(mm.ins, prev.ins, sync=False,
                                   reason="spin order")
                prev = mm
            spin_last = prev

            # --- input loads.  Ring FIFOs drain slowly (~45-90 GB/s), so
            # the one-hot is cut into need-ordered slices spread across the
            # three rings; per-ring arrival order matches need order.
            oh_t = []
            for si in range(len(obnd) - 1):
                w = obnd[si + 1] - obnd[si]
                t = const_p.tile([K, w * TILE], FP8, tag=f"oh{si}",
                                 name=f"oh{si}")
                oh_t.append(t)

            def loadp(t, dsl, engs):
                n = len(engs)
                step = -(-K // n)
                for i, eng in enumerate(engs):
                    p0, p1 = i * step, min((i + 1) * step, K)
                    eng.dma_start(out=t[p0:p1, :], in_=dsl[p0:p1, :])

            def oh_load(si, engs):
                a, b = obnd[si], obnd[si + 1]
                loadp(oh_t[si], d_oh.ap()[:, a * TILE:b * TILE], engs)

            oh_load(0, [nc.sync, nc.scalar, nc.gpsimd])
            t_table = load(d_table, [K, D], FP8, nc.sync)
            t_rhs = load(d_rhs, [MRK, 3 * D], BF16, nc.scalar)
            t_masksT = load(d_masksT, [MRK, ntiles["act"] * TILE], BF16,
                            nc.gpsimd)
            oh_load(1, [nc.sync])
            oh_load(2, [nc.scalar])
            t_gmat = load(d_gmat, [MRK, 3 * 32], BF16, nc.gpsimd)
            oh_load(3, [nc.gpsimd])
            t_featU = const_p.tile([TILE, nfeat], BF16, tag="featU")
            nc.sync.dma_start(out=t_featU[:64, :], in_=d_featU.ap()[:64, :])
            nc.scalar.dma_start(out=t_featU[64:, :],
                                in_=d_featU.ap()[64:, :])
            t_cls_pack = load(d_cls_pack, [MRK, 3 * TILE], BF16, nc.gpsimd)
            t_masks4 = load(d_masks4, [TILE, 4], F32, nc.gpsimd)
            oh_load(4, [nc.sync])
            oh_load(5, [nc.scalar])
            t_eye = load(d_eye, [TILE, TILE], BF16, nc.gpsimd)
            t_ctxT = load(d_ctxT, [MRK, ntiles["ctx"] * TILE], BF16,
                          nc.gpsimd)
            oh_load(6, [nc.sync])
            oh_load(7, [nc.scalar, nc.gpsimd])

            eps_t = const_p.tile([TILE, 1], F32, tag="eps")
            nc.vector.memset(eps_t, 1e-5)

            # --- engine busy model for drain balancing ---
            busy = {"A": 0.0, "V": 0.0}
            A_FIX, A_NS = 160.0, 0.95
            V_FIX, V_NS = 130.0, 1.05

            def pick(cols, v_extra=0.0):
                ca = busy["A"] + A_FIX + A_NS * cols
                cv = busy["V"] + V_FIX + V_NS * cols + v_extra
                if ca <= cv * 1.05:
                    busy["A"] = ca
                    return "A"
                busy["V"] = cv
                return "V"

            def charge(eng, cols):
                if eng == "A":
                    busy["A"] += A_FIX + A_NS * cols
                else:
                    busy["V"] += V_FIX + V_NS * cols

            def get_oh(t0):
                for si in range(len(obnd) - 1):
                    if t0 < obnd[si + 1]:
                        return oh_t[si][:, (t0 - obnd[si]) * TILE:]
                raise AssertionError(t0)

            def gather(po, oh_sl, gn, prev=None):
                for i in range(gn):
                    mm = nc.tensor.matmul(
                        po[:, i * D:(i + 1) * D],
                        lhsT=oh_sl[:, i * TILE:(i + 1) * TILE],
                        rhs=t_table, start=(i % 2 == 0), stop=True,
                        skip_group_check=True)
                    if prev is not None:
                        add_dep_helper(mm.ins, prev.ins, sync=False,
                                       reason="psum bank order")
                    prev = mm
                return prev

            # --- output slab / flush machinery (dual dtype: card tiles go
            # out fp8, mlp/cls tiles bf16) ---
            sbuf8 = {"tile": None, "t0": 0, "cols": 0}
            sbuf16 = {"tile": None, "t0": 0, "cols": 0}
            flush_rr = [0]
            FT8 = 2 * FT

            def _flush(sb, dten):
                if sb["tile"] is None:
                    return
                t, t0, cols = sb["tile"], sb["t0"], sb["cols"]
                sb["tile"] = None
                rings = [nc.gpsimd, nc.sync, nc.scalar]
                eng = rings[flush_rr[0] % 3]
                flush_rr[0] += 1
                eng.dma_start(out=dten.ap()[:, t0 * D:t0 * D + cols],
                              in_=t[:, :cols])

            def _slot(sb, dten, t0, ncols, cap, mk):
                if sb["tile"] is not None and (
                        sb["t0"] * D + sb["cols"] != t0 * D
                        or sb["cols"] + ncols > cap * D):
                    _flush(sb, dten)
                if sb["tile"] is None:
                    sb.update(tile=mk(), t0=t0, cols=0)
                off = sb["cols"]
                sb["cols"] += ncols
                return sb["tile"], off

            def store_slot8(t0, ncols):
                return _slot(sbuf8, d_out8, t0, ncols, FT8,
                             lambda: out_p.tile([TILE, FT8 * D], FP8,
                                                tag="o8", name="o8t"))

            def store_slot(t0, ncols):
                return _slot(sbuf16, d_out, t0, ncols, FT,
                             lambda: out_p.tile([TILE, FT * D], BF16,
                                                tag="o", name="o16t"))

            def maybe_flush():
                if sbuf8["tile"] is not None and sbuf8["cols"] >= FT8 * D:
                    _flush(sbuf8, d_out8)
                if sbuf16["tile"] is not None and sbuf16["cols"] >= FT * D:
                    _flush(sbuf16, d_out)

            def flush_store():
                _flush(sbuf8, d_out8)
                _flush(sbuf16, d_out)

            # --- per-kind drain / finish ops ---
            def copy_store(po, t0, gn):
                cols = gn * D
                o_sb, off = store_slot8(t0, cols)
                if pick(cols) == "A":
                    nc.scalar.activation(out=o_sb[:, off:off + cols],
                                         in_=po[:, :cols], func=AF.Copy)
                else:
                    nc.vector.tensor_copy(out=o_sb[:, off:off + cols],
                                          in_=po[:, :cols])
                maybe_flush()

            def mlp_mms(po, ph, phS, mms, prev_mm):
                for i, (lhsT_sl, rhs_w, rhs_g, _u) in enumerate(mms):
                    mm = nc.tensor.matmul(
                        ph[:, i * D:(i + 1) * D],
                        lhsT=lhsT_sl, rhs=rhs_w,
                        start=(i % 2 == 0), stop=True,
                        skip_group_check=True)
                    if prev_mm is not None:
                        add_dep_helper(mm.ins, prev_mm.ins, sync=False,
                                       reason="psum bank order")
                    mm2 = nc.tensor.matmul(
                        phS[:, i * 32:(i + 1) * 32],
                        lhsT=lhsT_sl, rhs=rhs_g,
                        start=(i == 0), stop=True,
                        skip_group_check=True)
                    add_dep_helper(mm2.ins, mm.ins, sync=False,
                                   reason="psum bank order")
                    prev_mm = mm2
                return prev_mm

            def mlp_var_rstd(phS, mms, vr, rr):
                """var = sum((x@G) * x) per tile, rr = 1/sqrt(var+eps)."""
                gn = len(mms)
                scr = small_p.tile([TILE, GRP * 32], F32, tag="scr")
                for i, (_l, _w, _g, ublock) in enumerate(mms):
                    kr = ublock[1]
                    nc.vector.scalar_tensor_tensor(
                        out=scr[:, i * 32:i * 32 + kr],
                        in0=phS[:, i * 32:i * 32 + kr], scalar=1.0,
                        in1=t_featU[:, ublock[0]:ublock[0] + kr],
                        op0=ALU.mult, op1=ALU.mult,
                        accum_out=vr[:, i:i + 1])
                    charge("V", kr)
                sd = small_p.tile([TILE, GRP], F32, tag="sd")
                nc.scalar.activation(out=sd[:, :gn], in_=vr[:, :gn],
                                     func=AF.Sqrt, bias=eps_t)
                busy["A"] += A_FIX + gn
                nc.vector.reciprocal(out=rr[:, :gn], in_=sd[:, :gn])
                busy["V"] += V_FIX + gn

            def mlp_relus(ph, gn, rsc):
                # plain relu (no scale) right after the h' matmuls, one op
                # per group; the rstd scale rides the stt's scalar AP
                cols = gn * D
                if pick(cols) == "A":
                    nc.scalar.activation(out=rsc[:, :cols],
                                         in_=ph[:, :cols], func=AF.Relu)
                else:
                    nc.vector.tensor_scalar(
                        out=rsc[:, :cols], in0=ph[:, :cols],
                        scalar1=0.0, scalar2=None, op0=ALU.max)

            def stage_b_mlp(t0, gn, po, rr, rsc):
                o_sb, off = store_slot(t0, gn * D)
                for i in range(gn):
                    # out = relu(h') * rstd + gather
                    nc.vector.scalar_tensor_tensor(
                        out=o_sb[:, off + i * D:off + (i + 1) * D],
                        in0=rsc[:, i * D:(i + 1) * D],
                        scalar=rr[:, i:i + 1],
                        in1=po[:, i * D:(i + 1) * D],
                        op0=ALU.mult, op1=ALU.add)
                    charge("V", D)
                maybe_flush()

            from collections import defaultdict
            tasks = defaultdict(list)
            onext = {"o8": 0, "o16": 0}

            for gi, (kind, gn, t0, st) in enumerate(sched):
                for fn in tasks.pop(gi, ()):
                    fn()
                oh_sl = get_oh(t0)
                po = po0 if gi == 0 else po_p.tile([TILE, GRP * D], F32,
                                                   tag="po")
                last_mm = gather(po, oh_sl, gn,
                                 prev=spin_last if gi == 0 else None)
                okey = "o8" if kind == "card" else "o16"
                oslot = onext[okey]
                onext[okey] += gn

                if kind == "card":
                    tasks[gi + 1].append(
                        lambda po=po, t0=oslot, gn=gn:
                        copy_store(po, t0, gn))
                elif kind in ("act", "ctx"):
                    lhsT = t_masksT if kind == "act" else t_ctxT
                    rhs_w = (t_rhs[:, :D] if kind == "act"
                             else t_rhs[:, D:2 * D])
                    rhs_g = (t_gmat[:, 0:32] if kind == "act"
                             else t_gmat[:, 32:64])
                    u_base = 0 if kind == "act" else ntiles["act"] * 32
                    ph = po[:, GRP_MLP * D:2 * GRP_MLP * D]
                    phS = ph_p.tile([TILE, GRP * 32], F32, tag="phS")
                    mms = [(lhsT[:, (st + i) * TILE:(st + i + 1) * TILE],
                            rhs_w, rhs_g, ((u_base + (st + i) * 32), MRK))
                           for i in range(gn)]
                    mlp_mms(po, ph, phS, mms, last_mm)
                    vr = small_p.tile([TILE, 8], F32, tag="vr")
                    rr = small_p.tile([TILE, GRP], F32, tag="rr")
                    rsc = rsc_p.tile([TILE, GRP_MLP * D], BF16, tag="rsc")
                    mlp_relus(ph, gn, rsc)
                    mlp_var_rstd(phS, mms, vr, rr)
                    tasks[gi + 1].append(
                        lambda t0=oslot, gn=gn, po=po, rr=rr,
                        rsc=rsc: stage_b_mlp(t0, gn, po, rr, rsc))
                else:  # cls
                    phS = ph_p.tile([TILE, GRP * 32], F32, tag="phS")
                    u0 = (ntiles["act"] + ntiles["ctx"]) * 32
                    mms = [
                        (t_cls_pack[:, 0:TILE], t_rhs[:, :D],
                         t_gmat[:, 0:32], (u0, MRK)),
                        (t_cls_pack[:, TILE:2 * TILE], t_rhs[:, D:2 * D],
                         t_gmat[:, 32:64], (u0 + 32, MRK)),
                        (t_cls_pack[0:4, 2 * TILE:3 * TILE],
                         t_rhs[0:4, 2 * D:3 * D],
                         t_gmat[0:4, 64:96], (u0 + 64, 4)),
                    ]
                    hsl = [slice(2 * D, 3 * D), slice(3 * D, 4 * D),
                           slice(D, 2 * D)]
                    prev_mm = last_mm
                    for i, (lhsT_sl, rhs_w, rhs_g, _u) in enumerate(mms):
                        mm = nc.tensor.matmul(
                            po[:, hsl[i]], lhsT=lhsT_sl, rhs=rhs_w,
                            start=(i == 0), stop=True,
                            skip_group_check=True)
                        add_dep_helper(mm.ins, prev_mm.ins, sync=False,
                                       reason="psum bank order")
                        mm2 = nc.tensor.matmul(
                            phS[:, i * 32:(i + 1) * 32],
                            lhsT=lhsT_sl, rhs=rhs_g,
                            start=(i == 0), stop=True,
                            skip_group_check=True)
                        add_dep_helper(mm2.ins, mm.ins, sync=False,
                                       reason="psum bank order")
                        prev_mm = mm2
                    vr = small_p.tile([TILE, 8], F32, tag="vr")
                    rr = small_p.tile([TILE, GRP], F32, tag="rr")
                    mlp_var_rstd(phS, mms, vr, rr)
                    mr = small_p.tile([TILE, 3], F32, tag="mr")
                    nc.vector.tensor_tensor(
                        out=mr[:, 0:3], in0=t_masks4[:, 0:3],
                        in1=rr[:, 0:3], op=ALU.mult)
                    busy["V"] += V_FIX + 3

                    relu_t = relu_p.tile([TILE, 3 * D], BF16, tag="relu")

                    def cls_relu(po=po, hsl=hsl, mr=mr, relu_t=relu_t):
                        for i in range(3):
                            # relu((mask*rstd) * h') = mask*rstd*relu(h')
                            if pick(D) == "A":
                                nc.scalar.activation(
                                    out=relu_t[:, i * D:(i + 1) * D],
                                    in_=po[:, hsl[i]], func=AF.Relu,
                                    scale=mr[:, i:i + 1])
                            else:
                                nc.vector.tensor_scalar(
                                    out=relu_t[:, i * D:(i + 1) * D],
                                    in0=po[:, hsl[i]],
                                    scalar1=mr[:, i:i + 1], scalar2=0.0,
                                    op0=ALU.mult, op1=ALU.max)

                    def cls_acc(po=po, relu_t=relu_t):
                        # accumulate the three relu terms onto the gather
                        # PSUM via identity matmuls
                        prev = None
                        for i in range(3):
                            mm = nc.tensor.matmul(
                                po[:, :D], lhsT=t_eye,
                                rhs=relu_t[:, i * D:(i + 1) * D],
                                start=False, stop=(i == 2),
                                skip_group_check=True)
                            if prev is not None:
                                add_dep_helper(mm.ins, prev.ins,
                                               sync=False,
                                               reason="psum acc order")
                            prev = mm

                    def cls_drain(t0=oslot, po=po):
                        o_sb, off = store_slot(t0, D)
                        if pick(D) == "A":
                            nc.scalar.activation(
                                out=o_sb[:, off:off + D], in_=po[:, :D],
                                func=AF.Copy, scale=t_masks4[:, 3:4])
                        else:
                            nc.vector.tensor_scalar(
                                out=o_sb[:, off:off + D], in0=po[:, :D],
                                scalar1=t_masks4[:, 3:4], scalar2=None,
                                op0=ALU.mult)
                        maybe_flush()

                    tasks[gi + 1].append(cls_relu)
                    tasks[gi + 2].append(cls_acc)
                    tasks[gi + 2].append(cls_drain)

            for i in sorted(tasks):
                for fn in tasks[i]:
                    fn()
            flush_store()

    if not nc.is_finalized():
        nc.finalize()
    return nc


def kernel(token_ids, token_streets, card_ranks, card_suits, action_actors,
           action_legal_masks, context_features,
           base_emb, street_emb, rank_emb, suit_emb, actor_emb, atype_emb,
           legal_W, legal_b, legal_g, legal_be,
           cls_W, cls_b, cls_g, cls_be,
           ctx_W, ctx_b, ctx_g, ctx_be, _trace=False):
    per_core, sched, nt, ntiles = _build_host_data(
        np.asarray(token_ids), np.asarray(token_streets),
        np.asarray(card_ranks), np.asarray(card_suits),
        np.asarray(action_actors), np.asarray(action_legal_masks),
        np.asarray(context_features))

    for g, be in ((legal_g, legal_be), (cls_g, cls_be), (ctx_g, ctx_be)):
        assert np.allclose(np.asarray(g), 1.0) and np.allclose(
            np.asarray(be), 0.0), "non-trivial LN affine not supported"

    t_all, rhs, gmat, eye = _build_tables(
        np.asarray(base_emb), np.asarray(street_emb), np.asarray(rank_emb),
        np.asarray(suit_emb), np.asarray(actor_emb), np.asarray(atype_emb),
        np.asarray(legal_W), np.asarray(legal_b), np.asarray(ctx_W),
        np.asarray(ctx_b), np.asarray(cls_W), np.asarray(cls_b))

    nc = _build_bass(sched, nt, ntiles)

    shared = dict(table=t_all, rhs=rhs, gmat=gmat, eye=eye)
    in_maps = []
    for c in range(NCORES):
        pc = per_core[c]
        im = dict(shared)
        im.update(oh=pc["oh"], masksT=pc["masksT"], ctxT=pc["ctxT"],
                  cls_pack=pc["cls_pack"], featU=pc["featU"],
                  masks4=pc["masks4"])
        in_maps.append(im)

    res = run_bass_kernel_spmd(nc, in_maps, core_ids=list(range(NCORES)),
                               trace=_trace)
    if _trace:
        print(f"HW exec time: {res.exec_time_ns} ns")
        print(f"mean exec time: {res.mean_exec_time_ns} ns")
        if res.instructions_and_trace:
            print("trace:", res.instructions_and_trace[1])

    # per-tile output mapping: card tiles -> (out8, slot), rest -> (out, slot)
    src8 = np.full(nt, -1, np.int64)
    src16 = np.full(nt, -1, np.int64)
    c8 = c16 = 0
    for kind, gn, t0, _ in sched:
        if kind == "card":
            src8[t0:t0 + gn] = np.arange(c8, c8 + gn)
            c8 += gn
        else:
            src16[t0:t0 + gn] = np.arange(c16, c16 + gn)
            c16 += gn

    full = np.zeros((B * S, D), np.float32)
    for c in range(NCORES):
        o16 = np.asarray(res.results[c]["out"]).astype(np.float32)
        o8 = np.asarray(res.results[c]["out8"]).astype(np.float32)
        o16 = o16.reshape(TILE, c16, D).transpose(1, 0, 2)
        o8 = o8.reshape(TILE, c8, D).transpose(1, 0, 2)
        rows = np.empty((nt, TILE, D), np.float32)
        rows[src8 >= 0] = o8[src8[src8 >= 0]]
        rows[src16 >= 0] = o16[src16[src16 >= 0]]
        rows = rows.reshape(-1, D)
        slots = per_core[c]["slots"]
        valid = slots >= 0
        full[slots[valid]] = rows[valid]
    return full.reshape(B, S, D)
